# Initial kernel scaffold
#
"""Multi-head attention (whisper-style, returns (out, qk)) on 8 Trainium2 cores.

Sharding: core c -> (batch b = c//2, head-group hg = c%2). Each core computes
8 heads (512 features) of one batch: QKV projections, causal attention scores
(returned as qk), softmax, attention-weighted V, and a partial output
projection. Host sums the two head-group partials per batch and adds bo.

All matmuls run in float32r (TF32-like fast path). Causal mask applied on
device via affine_select (-inf fill) for the stored qk; softmax path uses a
transposed score layout with triangular skipping.
"""

import sys

sys.path.insert(0, "/opt/trn_rl_repo")

import numpy as np

import concourse.bass as bass  # noqa: F401  (import registers AP machinery)
from concourse import bacc, bass_utils, mybir
import concourse.tile as tile

B, T, D, H = 4, 1500, 1024, 16
DH = D // H              # 64
NCORES = 8
HPC = H // 2             # 8 heads per core
FPC = HPC * DH           # 512 features per core
NT = (T + 127) // 128    # 12 partition tiles over T (last has 92 rows)
CW = 500                 # column chunk width (<=512 fp32 psum bank)
NJ = T // CW             # 3 chunks
VW = DH + 1              # v columns per head incl. ones column (65)

f32 = mybir.dt.float32
f32r = mybir.dt.float32r
EXP = mybir.ActivationFunctionType.Exp
IDENT = mybir.ActivationFunctionType.Identity
GE = mybir.AluOpType.is_ge

_cached_nc = None


def _trows(i):
    return min(128, T - 128 * i)


def _build():
    nc = bacc.Bacc("TRN2", target_bir_lowering=False, debug=False)

    xT = nc.dram_tensor("xT", [D + 1, T], f32r, kind="ExternalInput").ap()
    wqT = nc.dram_tensor("wqT", [D, FPC], f32r, kind="ExternalInput").ap()
    wkT = nc.dram_tensor("wkT", [D, FPC], f32r, kind="ExternalInput").ap()
    wvT = nc.dram_tensor("wvT", [D + 1, FPC], f32r, kind="ExternalInput").ap()
    woT = nc.dram_tensor("woT", [FPC, D], f32r, kind="ExternalInput").ap()
    bqv = nc.dram_tensor("bq", [FPC, 1], f32, kind="ExternalInput").ap()
    qk_out = nc.dram_tensor("qk_out", [HPC, T, T], f32, kind="ExternalOutput").ap()
    outT = nc.dram_tensor("outT", [D, T], f32, kind="ExternalOutput").ap()

    with tile.TileContext(nc) as tc:
        # ---------------- persistent SBUF ----------------
        with tc.tile_pool(name="perm", bufs=1) as perm:
            qT_sb = [perm.tile([128, T], f32r, name=f"qT{m}") for m in range(4)]
            kT_sb = [perm.tile([128, T], f32r, name=f"kT{m}") for m in range(4)]
            v_sb = [perm.tile([128, HPC * VW], f32r, name=f"v{i}") for i in range(NT)]
            oT_sb = [perm.tile([128, T], f32r, name=f"oT{m}") for m in range(4)]

            # ---------------- phase 1: QKV projections ----------------
            with tc.tile_pool(name="xw", bufs=1) as xw:
                xT_sb = [xw.tile([128, T], f32r, name=f"x{k}") for k in range(8)]
                for k in range(8):
                    nc.sync.dma_start(xT_sb[k][:], xT[128 * k : 128 * (k + 1), :])
                xT1_sb = xw.tile([1, T], f32r, name="x_ones")
                nc.sync.dma_start(xT1_sb[:], xT[D : D + 1, :])

                # q^T = (Wq_p*s) @ x^T + bq*s   (bias via ACT on evacuation)
                # k^T = (Wk_p*s) @ x^T
                with tc.tile_pool(name="wq", bufs=1) as wq, \
                     tc.tile_pool(name="qkps", bufs=2, space="PSUM") as qkps:
                    wq_sb = [wq.tile([128, FPC], f32r, name=f"wq{k}") for k in range(8)]
                    bq_sb = [wq.tile([128, 1], f32, name=f"bq{m}") for m in range(4)]
                    for k in range(8):
                        nc.sync.dma_start(wq_sb[k][:], wqT[128 * k : 128 * (k + 1), :])
                    for m in range(4):
                        nc.sync.dma_start(bq_sb[m][:], bqv[128 * m : 128 * (m + 1), :])
                    for m in range(4):
                        qp = qkps.tile([128, T], f32, tag="qkp")
                        for j in range(NJ):
                            for k in range(8):
                                nc.tensor.matmul(
                                    qp[:, CW * j : CW * (j + 1)],
                                    wq_sb[k][:, 128 * m : 128 * (m + 1)],
                                    xT_sb[k][:, CW * j : CW * (j + 1)],
                                    start=(k == 0), stop=(k == 7),
                                )
                        nc.scalar.activation(qT_sb[m][:], qp[:], IDENT, bias=bq_sb[m][:])

                    wk_sb = [wq.tile([128, FPC], f32r, name=f"wk{k}") for k in range(8)]
                    for k in range(8):
                        nc.sync.dma_start(wk_sb[k][:], wkT[128 * k : 128 * (k + 1), :])
                    for m in range(4):
                        kp = qkps.tile([128, T], f32, tag="qkp")
                        for j in range(NJ):
                            for k in range(8):
                                nc.tensor.matmul(
                                    kp[:, CW * j : CW * (j + 1)],
                                    wk_sb[k][:, 128 * m : 128 * (m + 1)],
                                    xT_sb[k][:, CW * j : CW * (j + 1)],
                                    start=(k == 0), stop=(k == 7),
                                )
                        nc.vector.tensor_copy(kT_sb[m][:], kp[:])

                # v = x @ Wv_p^T + bv  (bias via ones-row of xT / bv-row of wvT)
                # stored interleaved per head: [v_h (64) | 1] * 8  -> 520 cols
                with tc.tile_pool(name="wv", bufs=1) as wv, \
                     tc.tile_pool(name="vps", bufs=2, space="PSUM") as vps:
                    wv_sb = [wv.tile([128, FPC], f32r, name=f"wv{k}") for k in range(8)]
                    for k in range(8):
                        nc.sync.dma_start(wv_sb[k][:], wvT[128 * k : 128 * (k + 1), :])
                    wv1_sb = wv.tile([1, FPC], f32r, name="wv_bias")
                    nc.sync.dma_start(wv1_sb[:], wvT[D : D + 1, :])
                    for i in range(NT):
                        rw = _trows(i)
                        vp = vps.tile([128, FPC], f32, tag="vp")
                        for k in range(8):
                            nc.tensor.matmul(
                                vp[0:rw, :],
                                xT_sb[k][:, 128 * i : 128 * i + rw],
                                wv_sb[k][:],
                                start=(k == 0), stop=False,
                            )
                        nc.tensor.matmul(
                            vp[0:rw, :],
                            xT1_sb[:, 128 * i : 128 * i + rw],
                            wv1_sb[:],
                            start=False, stop=True,
                        )
                        nc.vector.tensor_copy(
                            v_sb[i][0:rw].rearrange("p (h c) -> p h c", c=VW)[:, :, 0:DH],
                            vp[0:rw].rearrange("p (h c) -> p h c", c=DH),
                        )
                        nc.vector.memset(
                            v_sb[i][0:rw].rearrange("p (h c) -> p h c", c=VW)[:, :, DH : DH + 1],
                            1.0,
                        )

            # ---------------- phase 2: attention per head ----------------
            with tc.tile_pool(name="sstage", bufs=3) as sstage, \
                 tc.tile_pool(name="et", bufs=6) as etp, \
                 tc.tile_pool(name="nrm", bufs=2) as nrm, \
                 tc.tile_pool(name="sps", bufs=2, space="PSUM") as sps, \
                 tc.tile_pool(name="stps", bufs=3, space="PSUM") as stps, \
                 tc.tile_pool(name="pops", bufs=1, space="PSUM") as pops:
                for h in range(HPC):
                    ht, fo = divmod(h, 2)
                    fo *= DH
                    qh = qT_sb[ht][fo : fo + DH, :]
                    kh = kT_sb[ht][fo : fo + DH, :]

                    # --- scores S[tq, tk] for the qk output (valid prefix) ---
                    for i in range(NT):
                        rw = _trows(i)
                        vend = min(128 * i + 128, T)     # cols [0, vend) written
                        st = sstage.tile([128, T], f32, tag="sstage")
                        nj = (vend + CW - 1) // CW
                        for j in range(nj):
                            ce = min(CW * (j + 1), vend)
                            sp = sps.tile([128, CW], f32, tag="sp")
                            nc.tensor.matmul(
                                sp[0:rw, :],
                                qh[:, 128 * i : 128 * i + rw],
                                kh[:, CW * j : CW * (j + 1)],
                                start=True, stop=True,
                            )
                            nc.vector.tensor_copy(
                                st[0:rw, CW * j : ce], sp[0:rw, 0 : ce - CW * j]
                            )
                        # causal -inf fill on the diagonal window [128i, vend)
                        nc.gpsimd.affine_select(
                            st[0:rw, 128 * i : vend],
                            st[0:rw, 128 * i : vend],
                            pattern=[[-1, vend - 128 * i]],
                            compare_op=GE, fill=float("-inf"),
                            base=0, channel_multiplier=1,
                        )
                        nc.sync.dma_start(
                            qk_out[h, 128 * i : 128 * i + rw, 0:vend], st[0:rw, 0:vend]
                        )

                    # --- transposed scores -> exp -> E^T, then out = E^T.T@v ---
                    po = pops.tile([VW, T], f32, tag="po")
                    et_tiles = {}
                    for m in range(NT):
                        rw = _trows(m)
                        c0 = 128 * m                     # valid tq suffix start
                        j0 = c0 // CW
                        for j in range(j0, NJ):
                            stp = stps.tile([128, CW], f32, tag="stp")
                            nc.tensor.matmul(
                                stp[0:rw, :],
                                kh[:, c0 : c0 + rw],
                                qh[:, CW * j : CW * (j + 1)],
                                start=True, stop=True,
                            )
                            et = etp.tile([128, CW], f32r, tag="et")
                            cs = max(c0, CW * j) - CW * j   # in-tile valid start
                            if cs > 0:
                                nc.vector.memset(et[0:rw, 0:cs], 0.0)
                            nc.scalar.activation(
                                et[0:rw, cs:CW], stp[0:rw, cs:CW], EXP
                            )
                            # zero the strictly-lower part of the diagonal window
                            wa = max(c0, CW * j)
                            wb = min(c0 + 128, CW * (j + 1))
                            if wa < wb:
                                nc.gpsimd.affine_select(
                                    et[0:rw, wa - CW * j : wb - CW * j],
                                    et[0:rw, wa - CW * j : wb - CW * j],
                                    pattern=[[1, wb - wa]],
                                    compare_op=GE, fill=0.0,
                                    base=wa - c0, channel_multiplier=-1,
                                )
                            et_tiles[(m, j)] = (et, rw)
                    for j in range(NJ):
                        ms = [m for m in range(NT) if 128 * m < CW * (j + 1)]
                        for n, m in enumerate(ms):
                            et, rw = et_tiles[(m, j)]
                            nc.tensor.matmul(
                                po[:, CW * j : CW * (j + 1)],
                                v_sb[m][0:rw, VW * h : VW * (h + 1)],
                                et[0:rw, :],
                                start=(n == 0), stop=(n == len(ms) - 1),
                            )

                    # --- normalize rows: out_h^T = po[0:64] * (1/po[64]) ---
                    dsb = nrm.tile([1, T], f32, tag="dsb")
                    nc.vector.tensor_copy(dsb[:], po[DH : DH + 1, :])
                    rsb = nrm.tile([1, T], f32, tag="rsb")
                    nc.vector.reciprocal(rsb[:], dsb[:])
                    rbc = nrm.tile([DH, T], f32, tag="rbc")
                    nc.gpsimd.partition_broadcast(rbc[:], rsb[:])
                    nc.vector.tensor_mul(
                        oT_sb[ht][fo : fo + DH, :], po[0:DH, :], rbc[:]
                    )

            # ---------------- phase 3: output projection ----------------
            with tc.tile_pool(name="wo", bufs=1) as wo, \
                 tc.tile_pool(name="ostage", bufs=2) as ostage, \
                 tc.tile_pool(name="ops", bufs=2, space="PSUM") as ops:
                wo_sb = [wo.tile([128, D], f32r, name=f"wo{k}") for k in range(4)]
                for k in range(4):
                    nc.sync.dma_start(wo_sb[k][:], woT[128 * k : 128 * (k + 1), :])
                for n in range(8):
                    pp = ops.tile([128, T], f32, tag="pp")
                    for j in range(NJ):
                        for k in range(4):
                            nc.tensor.matmul(
                                pp[:, CW * j : CW * (j + 1)],
                                wo_sb[k][:, 128 * n : 128 * (n + 1)],
                                oT_sb[k][:, CW * j : CW * (j + 1)],
                                start=(k == 0), stop=(k == 3),
                            )
                    ot = ostage.tile([128, T], f32, tag="ot")
                    nc.scalar.activation(ot[:], pp[:], IDENT)
                    nc.sync.dma_start(outT[128 * n : 128 * (n + 1), :], ot[:])

    nc.compile()
    return nc


def _get_nc():
    global _cached_nc
    if _cached_nc is None:
        _cached_nc = _build()
    return _cached_nc


def kernel(x, mask, Wq, bq, Wk, Wv, bv, Wo, bo, _run_kwargs=None):
    x = np.asarray(x, dtype=np.float32)
    Wq = np.asarray(Wq, dtype=np.float32)
    bq = np.asarray(bq, dtype=np.float32)
    Wk = np.asarray(Wk, dtype=np.float32)
    Wv = np.asarray(Wv, dtype=np.float32)
    bv = np.asarray(bv, dtype=np.float32)
    Wo = np.asarray(Wo, dtype=np.float32)
    bo = np.asarray(bo, dtype=np.float32)

    nc = _get_nc()
    s = float(DH) ** -0.25

    in_maps = []
    for c in range(NCORES):
        b, hg = divmod(c, 2)
        sl = slice(hg * FPC, (hg + 1) * FPC)
        xT_aug = np.empty((D + 1, T), np.float32)
        xT_aug[:D] = x[b].T
        xT_aug[D] = 1.0
        wvT_aug = np.empty((D + 1, FPC), np.float32)
        wvT_aug[:D] = Wv[sl].T
        wvT_aug[D] = bv[sl]
        in_maps.append({
            "xT": xT_aug,
            "wqT": np.ascontiguousarray((Wq[sl] * s).T),
            "wkT": np.ascontiguousarray((Wk[sl] * s).T),
            "wvT": wvT_aug,
            "woT": np.ascontiguousarray(Wo[:, sl].T),
            "bq": (bq[sl] * s).reshape(FPC, 1).astype(np.float32),
        })

    res = bass_utils.run_bass_kernel_spmd(
        nc, in_maps, core_ids=list(range(NCORES)), **(_run_kwargs or {})
    )

    out = np.empty((B, T, D), np.float32)
    qk = np.empty((B, H, T, T), np.float32)
    for b in range(B):
        r0 = res.results[2 * b]
        r1 = res.results[2 * b + 1]
        out[b] = r0["outT"].T + r1["outT"].T + bo
        for hg, r in ((0, r0), (1, r1)):
            for hl in range(HPC):
                h = hg * HPC + hl
                dst = qk[b, h]
                src = r["qk_out"][hl]
                for i in range(NT):
                    ra, rb = 128 * i, 128 * i + _trows(i)
                    vend = min(128 * i + 128, T)
                    dst[ra:rb, :vend] = src[ra:rb, :vend]
                    dst[ra:rb, vend:] = -np.inf
    if _run_kwargs is not None:
        return (out, qk), res
    return out, qk


# revision 6
# speedup vs baseline: 1.0609x; 1.0609x over previous
"""Multi-head attention (whisper-style, returns (out, qk)) on 8 Trainium2 cores.

Sharding: core c -> (batch b = c//2, head-group hg = c%2). Each core computes
8 heads (512 features) of one batch: QKV projections, causal attention scores
(returned as qk), softmax, attention-weighted V, and a partial output
projection. Host sums the two head-group partials per batch and adds bo.

All matmuls run in float32r (TF32-like fast path). Causal mask applied on
device via affine_select (-inf fill) for the stored qk; softmax path uses a
transposed score layout with triangular skipping.
"""

import sys

sys.path.insert(0, "/opt/trn_rl_repo")

import numpy as np

import concourse.bass as bass  # noqa: F401  (import registers AP machinery)
from concourse import bacc, bass_utils, mybir
import concourse.tile as tile

B, T, D, H = 4, 1500, 1024, 16
DH = D // H              # 64
NCORES = 8
HPC = H // 2             # 8 heads per core
FPC = HPC * DH           # 512 features per core
NT = (T + 127) // 128    # 12 partition tiles over T (last has 92 rows)
CW = 500                 # column chunk width (<=512 fp32 psum bank)
NJ = T // CW             # 3 chunks
VW = DH + 1              # v columns per head incl. ones column (65)

f32 = mybir.dt.float32
f32r = mybir.dt.float32r
EXP = mybir.ActivationFunctionType.Exp
IDENT = mybir.ActivationFunctionType.Identity
GE = mybir.AluOpType.is_ge

_cached_nc = None


def _trows(i):
    return min(128, T - 128 * i)


def _build():
    nc = bacc.Bacc("TRN2", target_bir_lowering=False, debug=False)

    xT = nc.dram_tensor("xT", [D + 1, T], f32r, kind="ExternalInput").ap()
    wqT = nc.dram_tensor("wqT", [D, FPC], f32r, kind="ExternalInput").ap()
    wkT = nc.dram_tensor("wkT", [D, FPC], f32r, kind="ExternalInput").ap()
    wvT = nc.dram_tensor("wvT", [D + 1, HPC * VW], f32r, kind="ExternalInput").ap()
    woT = nc.dram_tensor("woT", [FPC, D], f32r, kind="ExternalInput").ap()
    bqv = nc.dram_tensor("bq", [FPC, 1], f32, kind="ExternalInput").ap()
    qk_out = nc.dram_tensor("qk_out", [HPC, T, T], f32, kind="ExternalOutput").ap()
    outT = nc.dram_tensor("outT", [D, T], f32, kind="ExternalOutput").ap()

    with tile.TileContext(nc) as tc:
        # ---------------- persistent SBUF ----------------
        with tc.tile_pool(name="perm", bufs=1) as perm:
            qT_sb = [perm.tile([128, T], f32r, name=f"qT{m}") for m in range(4)]
            kT_sb = [perm.tile([128, T], f32r, name=f"kT{m}") for m in range(4)]
            v_sb = [perm.tile([128, HPC * VW], f32r, name=f"v{i}") for i in range(NT)]
            oT_sb = [perm.tile([128, T], f32r, name=f"oT{m}") for m in range(4)]

            # ---------------- phase 1: QKV projections ----------------
            with tc.tile_pool(name="xw", bufs=1) as xw:
                xT_sb = [xw.tile([128, T], f32r, name=f"x{k}") for k in range(8)]
                for k in range(8):
                    nc.sync.dma_start(xT_sb[k][:], xT[128 * k : 128 * (k + 1), :])
                xT1_sb = xw.tile([1, T], f32r, name="x_ones")
                nc.sync.dma_start(xT1_sb[:], xT[D : D + 1, :])

                # q^T = (Wq_p*s) @ x^T + bq*s   (bias via ACT on evacuation)
                # k^T = (Wk_p*s) @ x^T
                with tc.tile_pool(name="wq", bufs=1) as wq, \
                     tc.tile_pool(name="qkps", bufs=2, space="PSUM") as qkps:
                    wq_sb = [wq.tile([128, FPC], f32r, name=f"wq{k}") for k in range(8)]
                    bq_sb = [wq.tile([128, 1], f32, name=f"bq{m}") for m in range(4)]
                    for k in range(8):
                        nc.sync.dma_start(wq_sb[k][:], wqT[128 * k : 128 * (k + 1), :])
                    for m in range(4):
                        nc.sync.dma_start(bq_sb[m][:], bqv[128 * m : 128 * (m + 1), :])
                    for m in range(4):
                        qp = qkps.tile([128, T], f32, tag="qkp")
                        for j in range(NJ):
                            for k in range(8):
                                nc.tensor.matmul(
                                    qp[:, CW * j : CW * (j + 1)],
                                    wq_sb[k][:, 128 * m : 128 * (m + 1)],
                                    xT_sb[k][:, CW * j : CW * (j + 1)],
                                    start=(k == 0), stop=(k == 7),
                                )
                        nc.scalar.activation(qT_sb[m][:], qp[:], IDENT, bias=bq_sb[m][:])

                    wk_sb = [wq.tile([128, FPC], f32r, name=f"wk{k}") for k in range(8)]
                    for k in range(8):
                        nc.sync.dma_start(wk_sb[k][:], wkT[128 * k : 128 * (k + 1), :])
                    for m in range(4):
                        kp = qkps.tile([128, T], f32, tag="qkp")
                        for j in range(NJ):
                            for k in range(8):
                                nc.tensor.matmul(
                                    kp[:, CW * j : CW * (j + 1)],
                                    wk_sb[k][:, 128 * m : 128 * (m + 1)],
                                    xT_sb[k][:, CW * j : CW * (j + 1)],
                                    start=(k == 0), stop=(k == 7),
                                )
                        nc.vector.tensor_copy(kT_sb[m][:], kp[:])

                # v = x @ Wv_p^T + bv  (bias via ones-row of xT / bv-row of wvT)
                # wvT is pre-interleaved on host: per head [wv_h (64) | e] where
                # the extra column is 0 except 1.0 in the bias row -> v_sb gets
                # [v_h | 1] * 8 directly (520 cols, 2 psum banks).
                NV = HPC * VW
                with tc.tile_pool(name="wv", bufs=1) as wv, \
                     tc.tile_pool(name="vps", bufs=2, space="PSUM") as vps:
                    wv_sb = [wv.tile([128, NV], f32r, name=f"wv{k}") for k in range(8)]
                    for k in range(8):
                        nc.sync.dma_start(wv_sb[k][:], wvT[128 * k : 128 * (k + 1), :])
                    wv1_sb = wv.tile([1, NV], f32r, name="wv_bias")
                    nc.sync.dma_start(wv1_sb[:], wvT[D : D + 1, :])
                    for i in range(NT):
                        rw = _trows(i)
                        vp = vps.tile([128, NV], f32, tag="vp")
                        for ca, cb in ((0, 512), (512, NV)):
                            for k in range(8):
                                nc.tensor.matmul(
                                    vp[0:rw, ca:cb],
                                    xT_sb[k][:, 128 * i : 128 * i + rw],
                                    wv_sb[k][:, ca:cb],
                                    start=(k == 0), stop=False,
                                )
                            nc.tensor.matmul(
                                vp[0:rw, ca:cb],
                                xT1_sb[:, 128 * i : 128 * i + rw],
                                wv1_sb[:, ca:cb],
                                start=False, stop=True,
                            )
                        nc.vector.tensor_copy(v_sb[i][0:rw, :], vp[0:rw, :])

            # ---------------- phase 2: attention per head ----------------
            with tc.tile_pool(name="sstage", bufs=3) as sstage, \
                 tc.tile_pool(name="et", bufs=6) as etp, \
                 tc.tile_pool(name="nrm", bufs=2) as nrm, \
                 tc.tile_pool(name="sps", bufs=2, space="PSUM") as sps, \
                 tc.tile_pool(name="stps", bufs=3, space="PSUM") as stps, \
                 tc.tile_pool(name="pops", bufs=1, space="PSUM") as pops:
                for h in range(HPC):
                    ht, fo = divmod(h, 2)
                    fo *= DH
                    qh = qT_sb[ht][fo : fo + DH, :]
                    kh = kT_sb[ht][fo : fo + DH, :]

                    # --- scores S[tq, tk] for the qk output (valid prefix) ---
                    for i in range(NT):
                        rw = _trows(i)
                        vend = min(128 * i + 128, T)     # cols [0, vend) written
                        st = sstage.tile([128, T], f32, tag="sstage")
                        nj = (vend + CW - 1) // CW
                        for j in range(nj):
                            ce = min(CW * (j + 1), vend)
                            sp = sps.tile([128, CW], f32, tag="sp")
                            nc.tensor.matmul(
                                sp[0:rw, :],
                                qh[:, 128 * i : 128 * i + rw],
                                kh[:, CW * j : CW * (j + 1)],
                                start=True, stop=True,
                            )
                            nc.vector.tensor_copy(
                                st[0:rw, CW * j : ce], sp[0:rw, 0 : ce - CW * j]
                            )
                        # causal -inf fill on the diagonal window [128i, vend)
                        nc.gpsimd.affine_select(
                            st[0:rw, 128 * i : vend],
                            st[0:rw, 128 * i : vend],
                            pattern=[[-1, vend - 128 * i]],
                            compare_op=GE, fill=float("-inf"),
                            base=0, channel_multiplier=1,
                        )
                        nc.sync.dma_start(
                            qk_out[h, 128 * i : 128 * i + rw, 0:vend], st[0:rw, 0:vend]
                        )

                    # --- transposed scores -> exp -> E^T -> po += v_aug.T @ E^T ---
                    po = pops.tile([VW, T], f32, tag="po")
                    for j in range(NJ):
                        ms = [m for m in range(NT) if 128 * m < CW * (j + 1)]
                        for n, m in enumerate(ms):
                            rw = _trows(m)
                            c0 = 128 * m                 # valid tq suffix start
                            stp = stps.tile([128, CW], f32, tag="stp")
                            nc.tensor.matmul(
                                stp[0:rw, :],
                                kh[:, c0 : c0 + rw],
                                qh[:, CW * j : CW * (j + 1)],
                                start=True, stop=True,
                            )
                            et = etp.tile([128, CW], f32r, tag="et")
                            cs = max(c0 - CW * j, 0)     # in-tile valid start
                            nc.scalar.activation(
                                et[0:rw, cs:CW], stp[0:rw, cs:CW], EXP
                            )
                            # zero everything strictly below the diagonal
                            # (covers the unwritten [0, cs) prefix too:
                            # keep where tq - tk >= 0, else fill 0)
                            me = min(c0 + 128, CW * (j + 1)) - CW * j
                            if c0 + 128 > CW * j and me > 0:
                                nc.gpsimd.affine_select(
                                    et[0:rw, 0:me],
                                    et[0:rw, 0:me],
                                    pattern=[[1, me]],
                                    compare_op=GE, fill=0.0,
                                    base=CW * j - c0, channel_multiplier=-1,
                                )
                            nc.tensor.matmul(
                                po[:, CW * j : CW * (j + 1)],
                                v_sb[m][0:rw, VW * h : VW * (h + 1)],
                                et[0:rw, :],
                                start=(n == 0), stop=(n == len(ms) - 1),
                            )

                    # --- normalize rows: out_h^T = po[0:64] * (1/po[64]) ---
                    dsb = nrm.tile([1, T], f32, tag="dsb")
                    nc.vector.tensor_copy(dsb[:], po[DH : DH + 1, :])
                    rsb = nrm.tile([1, T], f32, tag="rsb")
                    nc.vector.reciprocal(rsb[:], dsb[:])
                    rbc = nrm.tile([DH, T], f32, tag="rbc")
                    nc.gpsimd.partition_broadcast(rbc[:], rsb[:])
                    nc.vector.tensor_mul(
                        oT_sb[ht][fo : fo + DH, :], po[0:DH, :], rbc[:]
                    )

            # ---------------- phase 3: output projection ----------------
            with tc.tile_pool(name="wo", bufs=1) as wo, \
                 tc.tile_pool(name="ostage", bufs=2) as ostage, \
                 tc.tile_pool(name="ops", bufs=2, space="PSUM") as ops:
                wo_sb = [wo.tile([128, D], f32r, name=f"wo{k}") for k in range(4)]
                for k in range(4):
                    nc.sync.dma_start(wo_sb[k][:], woT[128 * k : 128 * (k + 1), :])
                for n in range(8):
                    pp = ops.tile([128, T], f32, tag="pp")
                    for j in range(NJ):
                        for k in range(4):
                            nc.tensor.matmul(
                                pp[:, CW * j : CW * (j + 1)],
                                wo_sb[k][:, 128 * n : 128 * (n + 1)],
                                oT_sb[k][:, CW * j : CW * (j + 1)],
                                start=(k == 0), stop=(k == 3),
                            )
                    ot = ostage.tile([128, T], f32, tag="ot")
                    nc.scalar.activation(ot[:], pp[:], IDENT)
                    nc.sync.dma_start(outT[128 * n : 128 * (n + 1), :], ot[:])

    nc.compile()
    return nc


def _get_nc():
    global _cached_nc
    if _cached_nc is None:
        _cached_nc = _build()
    return _cached_nc


def kernel(x, mask, Wq, bq, Wk, Wv, bv, Wo, bo, _run_kwargs=None):
    x = np.asarray(x, dtype=np.float32)
    Wq = np.asarray(Wq, dtype=np.float32)
    bq = np.asarray(bq, dtype=np.float32)
    Wk = np.asarray(Wk, dtype=np.float32)
    Wv = np.asarray(Wv, dtype=np.float32)
    bv = np.asarray(bv, dtype=np.float32)
    Wo = np.asarray(Wo, dtype=np.float32)
    bo = np.asarray(bo, dtype=np.float32)

    nc = _get_nc()
    s = float(DH) ** -0.25

    in_maps = []
    for c in range(NCORES):
        b, hg = divmod(c, 2)
        sl = slice(hg * FPC, (hg + 1) * FPC)
        xT_aug = np.empty((D + 1, T), np.float32)
        xT_aug[:D] = x[b].T
        xT_aug[D] = 1.0
        wvT_aug = np.zeros((D + 1, HPC * VW), np.float32)
        wvt = Wv[sl].T
        for hl in range(HPC):
            wvT_aug[:D, hl * VW : hl * VW + DH] = wvt[:, hl * DH : (hl + 1) * DH]
            wvT_aug[D, hl * VW : hl * VW + DH] = bv[sl][hl * DH : (hl + 1) * DH]
            wvT_aug[D, hl * VW + DH] = 1.0
        in_maps.append({
            "xT": xT_aug,
            "wqT": np.ascontiguousarray((Wq[sl] * s).T),
            "wkT": np.ascontiguousarray((Wk[sl] * s).T),
            "wvT": wvT_aug,
            "woT": np.ascontiguousarray(Wo[:, sl].T),
            "bq": (bq[sl] * s).reshape(FPC, 1).astype(np.float32),
        })

    res = bass_utils.run_bass_kernel_spmd(
        nc, in_maps, core_ids=list(range(NCORES)), **(_run_kwargs or {})
    )

    out = np.empty((B, T, D), np.float32)
    qk = np.empty((B, H, T, T), np.float32)
    for b in range(B):
        r0 = res.results[2 * b]
        r1 = res.results[2 * b + 1]
        out[b] = r0["outT"].T + r1["outT"].T + bo
        for hg, r in ((0, r0), (1, r1)):
            for hl in range(HPC):
                h = hg * HPC + hl
                dst = qk[b, h]
                src = r["qk_out"][hl]
                for i in range(NT):
                    ra, rb = 128 * i, 128 * i + _trows(i)
                    vend = min(128 * i + 128, T)
                    dst[ra:rb, :vend] = src[ra:rb, :vend]
                    dst[ra:rb, vend:] = -np.inf
    if _run_kwargs is not None:
        return (out, qk), res
    return out, qk
